# revision 1
# baseline (speedup 1.0000x reference)
"""Trainium2 Bass kernel for nn_GAT_38989713113447 (3-layer dense GAT).

Sharding: 8 heads over 8 cores for the two inner GAT layers (pure head
parallelism, no communication).  The head-concat + output projection
commutes into a sum of per-head projections: Who = sum_k h1_k @ Wo[k],
so a small AllReduce of [64, N] replaces an AllGather of the full
[1024, N] concat.  The output attention layer is sharded over node rows
(384 rows/core); the per-core column slice of WhoT is selected via a
one-hot matmul against a per-core input (no AllToAll), keeping the SPMD
program identical on every core.  The final
[3072, 64] output is assembled host-side from the per-core row slices.

Math: exp(leakyrelu(s)) = max(exp(s), exp(alpha*s)) for alpha in (0,1),
so the [N,N] attention kernel needs one ACT Exp pass (per-partition bias
adds f2_j), one 4x-mode tensor_scalar (rank-1 term p_i*q_j), a 2x
tensor_tensor max, and a mask multiply (split DVE/GPSIMD for balance).
Attention lives transposed ([j, i], j on partitions) so the PE contracts
over j for both the aggregation matmul and the softmax denominator
(ones-matmul).  Masked entries are exact zeros via the mask multiply;
softmax max-subtraction is skipped (attention logits are O(1)).
"""

import sys

sys.path.insert(0, "/opt/trn_rl_repo")

from contextlib import ExitStack

import numpy as np
import ml_dtypes

import concourse.bass as bass  # noqa: F401
import concourse.bacc as bacc
import concourse.tile as tile
from concourse import mybir
from concourse.bass_utils import run_bass_kernel_spmd

N = 3072
F = 256
H = 8
D = 128          # H1 == H2
OUT = 64
ALPHA = 0.2
NCORES = 8
NJB = N // 128   # 24 attention j-blocks
HALF = N // 2    # i-dim half per PSUM residency
ISL = N // NCORES  # 384 output rows per core

FP32 = mybir.dt.float32
BF16 = mybir.dt.bfloat16
AF = mybir.ActivationFunctionType
ALU = mybir.AluOpType

def _chunks(total, step):
    return [(o, min(step, total - o)) for o in range(0, total, step)]


class Builder:
    def __init__(self, nc, tc, ctx):
        self.nc = nc
        self.tc = tc
        p = lambda name, bufs, space=None: ctx.enter_context(
            tc.tile_pool(name=name, bufs=bufs, **({"space": space} if space else {}))
        )
        self.state = p("state", 1)
        self.mask = p("mask", 10)
        self.work = p("work", 4)
        self.att = p("att", 8)
        self.ps_agg = p("ps_agg", 1, "PSUM")
        self.ps_rs = p("ps_rs", 1, "PSUM")
        self.ps_sm = p("ps_sm", 2, "PSUM")
        self.misc = p("misc", 1)
        self.psel = p("psel", 2)

    def ones_tile(self, shape, dtype, name):
        t = self.state.tile(shape, dtype, tag=name, name=name)
        self.nc.vector.memset(t[:, :], 1.0)
        return t

    def bcast_row(self, row_ap, width, tag, exp_scale=None):
        """[1, width] bf16 SBUF row -> [128, width] bf16 tile via a DMA with
        a partition-step-0 source AP (reads the row 128x) -- no PSUM
        round-trip.  With exp_scale the row is first mapped through
        Exp(scale*x) on ACT (1-lane, cheap)."""
        nc = self.nc
        src = row_ap
        if exp_scale is not None:
            er = self.state.tile([1, width], BF16, tag=tag + "_row",
                                 name=tag + "_row")
            for off, w in _chunks(width, 512):
                nc.scalar.activation(er[:, off : off + w],
                                     row_ap[:, off : off + w], AF.Exp,
                                     scale=exp_scale)
            src = er
        if not hasattr(self, "_row_dram"):
            self._row_dram = {}
        if tag not in self._row_dram:
            self._row_dram[tag] = nc.dram_tensor(
                f"rowd_{tag}", [1, N], BF16)
        rd = self._row_dram[tag]
        out = self.state.tile([128, width], BF16, tag=tag)
        for off, w in _chunks(width, 512):
            nc.sync.dma_start(rd[:, off : off + w], src[:, off : off + w])
            rs_ap = rd[:1, off : off + w]
            bsrc = bass.AP(rs_ap.tensor, rs_ap.offset,
                           [[0, 128]] + [list(p) for p in rs_ap.ap])
            nc.sync.dma_start(out[:, off : off + w], bsrc)
        return out

    def project(self, hT_tiles, w_sb, dT, rank1=None, want_whT=True,
                jb_range=None):
        """WhT[d, i] (feature-major) and node-major Wh_nm[jb][j, dT].

        hT_tiles: list of [128, N] bf16 (partitions = features).
        w_sb: matching list of [128, dT] bf16 weight tiles.
        rank1=(seed_row [1,N], thw_row [1,dT]): adds seed_i * thw_d to the
        projection (the mergeState theta-add folded through W: (x+s*th)W =
        xW + s*(thW)) as one extra K=1 matmul per accumulation group."""
        nc = self.nc
        nk = len(hT_tiles)
        whT = None
        if want_whT:
            whT = self.state.tile([dT, N], BF16, tag="whT")
        for off, w in (_chunks(N, 512) if want_whT else []):
            ps = self.ps_sm.tile([dT, 512], FP32, tag="sm", name="sm")
            last_w = nk - 1 if rank1 is None else nk
            for k in range(nk):
                nc.tensor.matmul(
                    ps[:, :w], w_sb[k][:, :], hT_tiles[k][:, off : off + w],
                    start=(k == 0), stop=(k == last_w),
                )
            if rank1 is not None:
                seed_row, thw_row = rank1
                nc.tensor.matmul(
                    ps[:, :w], thw_row[:, :dT], seed_row[:, off : off + w],
                    start=False, stop=True,
                )
            nc.scalar.activation(whT[:, off : off + w], ps[:, :w], AF.Copy)
        wh_nm = []
        per = max(1, 512 // dT)  # jb blocks packed per PSUM tile
        jlo, jhi = jb_range if jb_range is not None else (0, NJB)
        for jb0 in range(jlo, jhi, per):
            njb = min(per, jhi - jb0)
            ps = self.ps_sm.tile([128, 512], FP32, tag="sm", name="sm")
            last_w = nk - 1 if rank1 is None else nk
            for u in range(njb):
                jb = jb0 + u
                for k in range(nk):
                    nc.tensor.matmul(
                        ps[:, u * dT : (u + 1) * dT],
                        hT_tiles[k][:, jb * 128 : (jb + 1) * 128],
                        w_sb[k][:, :],
                        start=(k == 0), stop=(k == last_w),
                    )
                if rank1 is not None:
                    seed_row, thw_row = rank1
                    nc.tensor.matmul(
                        ps[:, u * dT : (u + 1) * dT],
                        seed_row[:, jb * 128 : (jb + 1) * 128],
                        thw_row[:, :dT],
                        start=False, stop=True,
                    )
            t = self.state.tile([128, per * dT], BF16,
                                tag=f"whnm_{jb0 // per}", name="whnm")
            nc.scalar.activation(t[:, : njb * dT], ps[:, : njb * dT], AF.Copy)
            for u in range(njb):
                wh_nm.append(t[:, u * dT : (u + 1) * dT])
        return whT, wh_nm

    def f2_from_h(self, hT_tiles, wa2_sb, corr=None):
        """f2col/qcol via f2 = h @ (W a2); corr=(seed_row, c2_tile) adds
        c2*seed_j (layer-0 theta fold)."""
        nc = self.nc
        nk = len(hT_tiles)
        f2ps = self.ps_sm.tile([128, NJB], FP32, tag="sm", name="sm")
        for jb in range(NJB):
            last = nk - 1 if corr is None else nk
            for k in range(nk):
                nc.tensor.matmul(
                    f2ps[:, jb : jb + 1],
                    hT_tiles[k][:, jb * 128 : (jb + 1) * 128],
                    wa2_sb[k][:, :],
                    start=(k == 0), stop=(k == last),
                )
            if corr is not None:
                seed_row, c2_tile = corr
                nc.tensor.matmul(
                    f2ps[:, jb : jb + 1],
                    seed_row[:, jb * 128 : (jb + 1) * 128],
                    c2_tile[:, :],
                    start=False, stop=True,
                )
        f2col = self.state.tile([128, NJB], FP32, tag="f2col")
        nc.scalar.activation(f2col[:, :], f2ps[:, :], AF.Copy)
        qcol = self.state.tile([128, NJB], FP32, tag="qcol")
        nc.scalar.activation(qcol[:, :], f2ps[:, :], AF.Exp, scale=ALPHA)
        return f2col, qcol

    def f1_from_h(self, hT_tiles, wa1_sb, width, corr=None):
        """f1bc/p_bc via f1 = h @ (W a1); corr=(seed_row, c1_tile)."""
        nc = self.nc
        nk = len(hT_tiles)
        f1row = self.state.tile([1, width], BF16, tag="f1row")
        for off, w in _chunks(width, 512):
            ps = self.ps_sm.tile([1, 512], FP32, tag="sm", name="sm")
            last = nk - 1 if corr is None else nk
            for k in range(nk):
                nc.tensor.matmul(
                    ps[:, :w], wa1_sb[k][:, :],
                    hT_tiles[k][:, off : off + w],
                    start=(k == 0), stop=(k == last),
                )
            if corr is not None:
                seed_row, c1_tile = corr
                nc.tensor.matmul(
                    ps[:, :w], c1_tile[:, :], seed_row[:, off : off + w],
                    start=False, stop=True,
                )
            nc.scalar.activation(f1row[:, off : off + w], ps[:, :w], AF.Copy)
        f1bc = self.bcast_row(f1row, width, "f1bc")
        p_bc = self.bcast_row(f1row, width, "p_bc", exp_scale=ALPHA)
        return f1bc, p_bc

    def _rowd(self, tag, width):
        if not hasattr(self, "_row_dram"):
            self._row_dram = {}
        if tag not in self._row_dram:
            self._row_dram[tag] = self.nc.dram_tensor(
                f"rowd_{tag}", [1, width], BF16)
        return self._row_dram[tag]

    def _bcast_chunk(self, src_row, out_tile, tag, o, w):
        """Broadcast src_row[:, o:o+w] -> out_tile[128, o:o+w] via DRAM."""
        nc = self.nc
        rd = self._rowd(tag, N)
        nc.sync.dma_start(rd[:, o : o + w], src_row[:, o : o + w])
        rs_ap = rd[:1, o : o + w]
        bsrc = bass.AP(rs_ap.tensor, rs_ap.offset,
                       [[0, 128]] + [list(p) for p in rs_ap.ap])
        nc.sync.dma_start(out_tile[:, o : o + w], bsrc)

    def f2_part(self, hT, wa2, f2col, qcol, jlo, jhi):
        """Columns [jlo, jhi) of f2col/qcol from h0T blocks (layer 1)."""
        nc = self.nc
        njb = jhi - jlo
        ps = self.ps_sm.tile([128, NJB], FP32, tag="sm", name="sm")
        for jb in range(jlo, jhi):
            nc.tensor.matmul(
                ps[:, jb - jlo : jb - jlo + 1],
                hT[:, jb * 128 : (jb + 1) * 128], wa2[:, :],
                start=True, stop=True,
            )
        nc.scalar.activation(f2col[:, jlo:jhi], ps[:, :njb], AF.Copy)
        nc.scalar.activation(qcol[:, jlo:jhi], ps[:, :njb], AF.Exp,
                             scale=ALPHA)

    def f1_part(self, hT, wa1, f1row, erow, f1bc, p_bc, lo, hi, tag):
        """f1bc/p_bc broadcast columns [lo, hi) from h0T (layer 1)."""
        nc = self.nc
        for off, w in _chunks(hi - lo, 512):
            o = lo + off
            ps = self.ps_sm.tile([1, 512], FP32, tag="sm", name="sm")
            nc.tensor.matmul(ps[:, :w], wa1[:, :], hT[:, o : o + w],
                             start=True, stop=True)
            nc.scalar.activation(f1row[:, o : o + w], ps[:, :w], AF.Copy)
            nc.scalar.activation(erow[:, o : o + w], ps[:, :w], AF.Exp,
                                 scale=ALPHA)
            self._bcast_chunk(f1row, f1bc, tag, o, w)
            self._bcast_chunk(erow, p_bc, tag + "p", o, w)

    def f2_vectors(self, whT, a2_sb, dT):
        """f2col [128, NJB] fp32 and qcol = exp(alpha*f2col)."""
        nc = self.nc
        f2ps = self.ps_sm.tile([128, NJB], FP32, tag="sm", name="sm")
        for jb in range(NJB):
            nc.tensor.matmul(
                f2ps[:, jb : jb + 1],
                whT[:, jb * 128 : (jb + 1) * 128],
                a2_sb[:, :],
                start=True, stop=True,
            )
        f2col = self.state.tile([128, NJB], FP32, tag="f2col")
        nc.scalar.activation(f2col[:, :], f2ps[:, :], AF.Copy)
        qcol = self.state.tile([128, NJB], FP32, tag="qcol")
        nc.scalar.activation(qcol[:, :], f2ps[:, :], AF.Exp, scale=ALPHA)
        return f2col, qcol

    def f1_vectors(self, whT_i, a1_sb, width):
        """f1bc [128, width] bf16 (broadcast f1) and p_bc = exp(alpha*f1)."""
        nc = self.nc
        f1row = self.state.tile([1, width], BF16, tag="f1row")
        for off, w in _chunks(width, 512):
            ps = self.ps_sm.tile([1, 512], FP32, tag="sm", name="sm")
            nc.tensor.matmul(
                ps[:, :w], a1_sb[:, :], whT_i[:, off : off + w],
                start=True, stop=True,
            )
            nc.scalar.activation(f1row[:, off : off + w], ps[:, :w], AF.Copy)
        f1bc = self.bcast_row(f1row, width, "f1bc")
        p_bc = self.bcast_row(f1row, width, "p_bc", exp_scale=ALPHA)
        return f1bc, p_bc

    def attention_agg(self, mask_dram, f2col, qcol, f1bc, p_bc, wh_nm, dT,
                      width, h_out, out_elu, fold_rowsum, between=None):
        """Masked softmax + aggregation + normalize (+ELU) into h_out.

        between: callback invoked after the first i-half's emission; must
        only emit work whose inputs are fully emitted by then."""
        nc = self.nc
        half_w = min(width, HALF)
        for h0 in range(0, width, half_w):
            if h0 > 0 and between is not None:
                between()
                between = None
            hw = min(half_w, width - h0)
            nch = len(_chunks(hw, 512))
            arows = dT + 1 if fold_rowsum else dT
            agg_ps = [self.ps_agg.tile([arows, 512], FP32, tag=f"agg{ci}",
                                       name=f"agg{ci}")
                      for ci in range(nch)]
            rs_ps = None
            if not fold_rowsum:
                rs_ps = [self.ps_rs.tile([1, 512], FP32, tag=f"rs{ci}",
                                         name=f"rs{ci}")
                         for ci in range(nch)]
            md = mask_dram[h0 // half_w] if isinstance(mask_dram, list) \
                else mask_dram
            mo = 0 if isinstance(mask_dram, list) else h0
            for jb in range(NJB):
                mt = self.mask.tile([128, hw], BF16, tag="mask")
                nc.sync.dma_start(
                    mt[:, :],
                    md[jb * 128 : (jb + 1) * 128, mo : mo + hw],
                )
                e1 = self.work.tile([128, hw], BF16, tag="e1")
                nc.scalar.activation(
                    e1[:, :], f1bc[:, h0 : h0 + hw], AF.Exp,
                    bias=f2col[:, jb : jb + 1],
                )
                # T = max(p_i*q_j, E1) == exp(leakyrelu(f1+f2)) in ONE fused
                # DVE pass (scalar_tensor_tensor); A = T * mask on DVE too.
                # GPSIMD is kept idle: HW-measured, any GPSIMD share of this
                # chain costs far more than the DVE passes it saves.
                tt = self.work.tile([128, hw], BF16, tag="tt")
                nc.vector.scalar_tensor_tensor(
                    tt[:, :], p_bc[:, h0 : h0 + hw], qcol[:, jb : jb + 1],
                    e1[:, :], ALU.mult, ALU.max,
                )
                at = self.att.tile([128, hw], BF16, tag="at")
                nc.vector.tensor_tensor(at[:, :], tt[:, :], mt[:, :], ALU.mult)
                for ci, (off, w) in enumerate(_chunks(hw, 512)):
                    nc.tensor.matmul(
                        agg_ps[ci][:, :w], wh_nm[jb],
                        at[:, off : off + w],
                        start=(jb == 0), stop=(jb == NJB - 1),
                    )
                if rs_ps is not None:
                    for ci, (off, w) in enumerate(_chunks(hw, 512)):
                        nc.tensor.matmul(
                            rs_ps[ci][:, :w], self.ones128[:, :],
                            at[:, off : off + w],
                            start=(jb == 0), stop=(jb == NJB - 1),
                        )
            # softmax denominator -> reciprocal -> broadcast -> normalize
            rinv = self.misc.tile([1, hw], FP32, tag="rinv")
            for ci, (off, w) in enumerate(_chunks(hw, 512)):
                src = (agg_ps[ci][dT : dT + 1, :w] if fold_rowsum
                       else rs_ps[ci][:, :w])
                nc.vector.reciprocal_approx_fast(rinv[:, off : off + w], src)
            rb_sb = self.misc.tile([dT, hw], FP32, tag="rb_sb")
            for off, w in _chunks(hw, 512):
                ps = self.ps_sm.tile([dT, 512], FP32, tag="sm", name="sm")
                nc.tensor.matmul(
                    ps[:, :w], self.ones1f[:, :dT], rinv[:, off : off + w],
                    start=True, stop=True,
                )
                nc.scalar.activation(rb_sb[:, off : off + w], ps[:, :w], AF.Copy)
            hpn = self.misc.tile([dT, hw], FP32, tag="hpn")
            for ci, (off, w) in enumerate(_chunks(hw, 512)):
                nc.vector.tensor_tensor(
                    hpn[:, off : off + w], agg_ps[ci][:dT, :w],
                    rb_sb[:, off : off + w], ALU.mult,
                )
            if out_elu:
                # ELU(x) = exp(min(x,0)) - 1 + max(x,0)
                m = self.misc.tile([dT, hw], FP32, tag="elu_m")
                nc.vector.tensor_scalar(m[:, :], hpn[:, :], 0.0, None, ALU.min)
                e = self.misc.tile([dT, hw], FP32, tag="elu_e")
                nc.scalar.activation(e[:, :], m[:, :], AF.Exp)
                r = self.misc.tile([dT, hw], FP32, tag="elu_m")
                nc.vector.tensor_scalar(
                    r[:, :], hpn[:, :], 0.0, -1.0, ALU.max, ALU.add
                )
                nc.vector.tensor_add(h_out[:, h0 : h0 + hw], e[:, :], r[:, :])
            else:
                nc.vector.tensor_copy(h_out[:, h0 : h0 + hw], hpn[:, :])


def build(dbg=False, sim=False):
    nc = bacc.Bacc("TRN2", target_bir_lowering=False, num_devices=NCORES)

    xT = nc.dram_tensor("xT", [F, N], BF16, kind="ExternalInput")
    seed = nc.dram_tensor("seed", [1, N], BF16, kind="ExternalInput")
    theta = nc.dram_tensor("theta", [F, 1], BF16, kind="ExternalInput")
    adjT0 = nc.dram_tensor("adjT0", [N, HALF], BF16, kind="ExternalInput")
    adjT1 = nc.dram_tensor("adjT1", [N, HALF], BF16, kind="ExternalInput")
    adjT_osl = nc.dram_tensor("adjT_osl", [N, ISL], BF16, kind="ExternalInput")
    w0 = nc.dram_tensor("w0", [F, D], BF16, kind="ExternalInput")
    a01 = nc.dram_tensor("a01", [F, 1], BF16, kind="ExternalInput")
    a02 = nc.dram_tensor("a02", [F, 1], BF16, kind="ExternalInput")
    c01 = nc.dram_tensor("c01", [1, 2], BF16, kind="ExternalInput")
    w1 = nc.dram_tensor("w1", [D, D], BF16, kind="ExternalInput")
    a11 = nc.dram_tensor("a11", [D, 1], BF16, kind="ExternalInput")
    a12 = nc.dram_tensor("a12", [D, 1], BF16, kind="ExternalInput")
    wo = nc.dram_tensor("wo", [D, OUT], BF16, kind="ExternalInput")
    ident = nc.dram_tensor("ident", [128, 128], BF16, kind="ExternalInput")
    ao1 = nc.dram_tensor("ao1", [OUT, 1], BF16, kind="ExternalInput")
    ao2 = nc.dram_tensor("ao2", [OUT, 1], BF16, kind="ExternalInput")

    outT = nc.dram_tensor("outT", [OUT, ISL], FP32, kind="ExternalOutput")
    if dbg:
        h0_dbg = nc.dram_tensor("h0_dbg", [D, N], BF16, kind="ExternalOutput")
        h1_dbg = nc.dram_tensor("h1_dbg", [D, N], BF16, kind="ExternalOutput")
        whto_dbg = nc.dram_tensor("whto_dbg", [OUT, N], BF16, kind="ExternalOutput")
        whtsl_dbg = nc.dram_tensor("whtsl_dbg", [OUT, ISL], BF16, kind="ExternalOutput")
        f2o_dbg = nc.dram_tensor("f2o_dbg", [128, NJB], FP32, kind="ExternalOutput")
        oraw_dbg = nc.dram_tensor("oraw_dbg", [OUT, ISL], FP32, kind="ExternalOutput")

    ar_in = nc.dram_tensor("ar_in", [OUT, N], BF16)
    ar_out = nc.dram_tensor("ar_out", [OUT, N], BF16, addr_space="Shared")
    psel = nc.dram_tensor("psel", [N, ISL], BF16, kind="ExternalInput")

    with tile.TileContext(nc) as tc, ExitStack() as ctx:
        b = Builder(nc, tc, ctx)
        b.ones1 = b.ones_tile([1, 128], BF16, "ones1")
        b.ones1f = b.ones_tile([1, 128], FP32, "ones1f")
        b.ones128 = b.ones_tile([128, 1], BF16, "ones128")

        def load_w(ap, shape, tag, dt=BF16):
            s = b.state.tile(shape, dt, tag=tag, name=tag)
            nc.sync.dma_start(s[:, :], ap)
            return s

        theta_col = [load_w(theta[k * 128 : (k + 1) * 128, :], [128, 1],
                            f"theta{k}") for k in range(F // 128)]
        seed_sb = load_w(seed[:, :], [1, N], "seed")
        w0_sb = [load_w(w0[k * 128 : (k + 1) * 128, :], [128, D], f"w0_{k}")
                 for k in range(F // 128)]
        wa01_sb = [load_w(a01[k * 128 : (k + 1) * 128, :], [128, 1],
                          f"wa01_{k}") for k in range(F // 128)]
        wa02_sb = [load_w(a02[k * 128 : (k + 1) * 128, :], [128, 1],
                          f"wa02_{k}") for k in range(F // 128)]
        c01_sb = load_w(c01[:, :], [1, 2], "c01")
        w1_sb = [load_w(w1[:, :], [D, D], "w1")]
        wa11_sb = [load_w(a11[:, :], [D, 1], "wa11")]
        wa12_sb = [load_w(a12[:, :], [D, 1], "wa12")]
        wo_sb = [load_w(wo[:, :], [D, OUT], "wo")]
        ident_sb = load_w(ident[:, :], [128, 128], "ident")
        ao1_sb = load_w(ao1[:, :], [OUT, 1], "ao1")
        ao2_sb = load_w(ao2[:, :], [OUT, 1], "ao2")

        # ---- layer 0 ----
        # mergeState folds through W0: (x + seed*theta) @ W0 =
        # x@W0 + seed x (theta@W0); no hT materialization needed.
        xT_sb = []
        for fb in range(F // 128):
            t = b.state.tile([128, N], BF16, tag=f"hT{fb}", name="xTsb")
            nc.sync.dma_start(t[:, :], xT[fb * 128 : (fb + 1) * 128, :])
            xT_sb.append(t)
        thw_ps = b.ps_sm.tile([1, D], FP32, tag="sm", name="sm")
        for k in range(F // 128):
            nc.tensor.matmul(
                thw_ps[:, :], theta_col[k][:, :],
                w0_sb[k][:, :], start=(k == 0), stop=(k == F // 128 - 1),
            )
        thw_row = b.state.tile([1, D], BF16, tag="thw_row", name="thw_row")
        nc.scalar.activation(thw_row[:, :], thw_ps[:, :], AF.Copy)
        _, whnm0 = b.project(xT_sb, w0_sb, D, rank1=(seed_sb, thw_row),
                             want_whT=False)
        f2c0, qc0 = b.f2_from_h(xT_sb, wa02_sb,
                                corr=(seed_sb, c01_sb[:, 1:2]))
        f1b0, pb0 = b.f1_from_h(xT_sb, wa01_sb, N,
                                corr=(seed_sb, c01_sb[:, 0:1]))
        h0T = b.state.tile([D, N], BF16, tag="h0T")
        l1_pre = {}

        def emit_l1_early():
            # Safe early emission: jb 0-11 read only h0T columns < HALF,
            # whose writes are already emitted (round-5 lesson: Tile orders
            # an earlier-emitted read BEFORE a later-emitted write, so
            # emitting reads of not-yet-emitted ranges reads stale data).
            _, l1_pre["whnm_a"] = b.project([h0T], w1_sb, D, want_whT=False,
                                            jb_range=(0, NJB // 2))

        b.attention_agg([adjT0, adjT1], f2c0, qc0, f1b0, pb0, whnm0, D, N,
                        h0T, out_elu=True, fold_rowsum=False,
                        between=emit_l1_early)

        # ---- layer 1 ----
        _, whnm1_b = b.project([h0T], w1_sb, D, want_whT=False,
                               jb_range=(NJB // 2, NJB))
        whnm1 = l1_pre["whnm_a"] + whnm1_b
        f2c1, qc1 = b.f2_from_h([h0T], wa12_sb)
        f1b1, pb1 = b.f1_from_h([h0T], wa11_sb, N)
        h1T = b.state.tile([D, N], BF16, tag="h1T")
        who_part = b.state.tile([OUT, N], BF16, tag="who_part")

        def _who_chunk(off, w):
            ps = b.ps_sm.tile([OUT, 512], FP32, tag="sm", name="sm")
            nc.tensor.matmul(
                ps[:, :w], wo_sb[0][:, :], h1T[:, off : off + w],
                start=True, stop=True,
            )
            nc.scalar.activation(who_part[:, off : off + w], ps[:, :w],
                                 AF.Copy)
            nc.sync.dma_start(ar_in[:, off : off + w],
                              who_part[:, off : off + w])

        def emit_who_early():
            # h1T cols < HALF are fully emitted after half 0 (round-5/6
            # rule); stage the first half of the AllReduce input early.
            for off, w in _chunks(HALF, 512):
                _who_chunk(off, w)

        b.attention_agg([adjT0, adjT1], f2c1, qc1, f1b1, pb1, whnm1, D, N,
                        h1T, out_elu=True, fold_rowsum=False,
                        between=emit_who_early)

        if dbg:
            nc.sync.dma_start(h0_dbg[:, :], h0T[:, :])
            nc.sync.dma_start(h1_dbg[:, :], h1T[:, :])

        # ---- out layer via AllReduce of per-head Who contributions ----
        # Who = hc @ Wo = sum_k h1_k @ Wo[k-block]; each core computes its
        # head's [64, N] share locally; AllReduce(add) sums over heads.
        # (A split early-launched AllReduce measured SLOWER on HW: the
        # mid-attention rendezvous + DMA contention outweigh tail overlap.)
        for off, w in _chunks(N, 512):
            if off < HALF:
                continue  # emitted early between layer-1 halves
            _who_chunk(off, w)
        if sim:
            nc.sync.dma_start(ar_out[:, :], ar_in[:, :])
        else:
            nc.gpsimd.collective_compute(
                "AllReduce", ALU.add,
                replica_groups=[list(range(NCORES))],
                ins=[ar_in.ap().opt()], outs=[ar_out.ap().opt()],
            )
        whTo = b.state.tile([OUT, N], BF16, tag="whTo")
        nc.sync.dma_start(whTo[:, :], ar_out[:, :])
        # node-major Who via PE transpose of WhoT 128-col chunks
        whnmo = []
        for jb in range(NJB):
            ps = b.ps_sm.tile([128, OUT], BF16, tag="sm", name="sm")
            nc.tensor.transpose(
                ps[:, :], whTo[:, jb * 128 : (jb + 1) * 128],
                ident_sb[:OUT, :OUT],
            )
            t = b.state.tile([128, OUT], BF16, tag=f"whnm_{jb}", name="whnm")
            nc.vector.tensor_copy(t[:, :], ps[:, :])
            whnmo.append(t)
        f2co, qco = b.f2_vectors(whTo, ao2_sb, OUT)
        if dbg:
            nc.sync.dma_start(whto_dbg[:, :], whTo[:, :])
            nc.sync.dma_start(f2o_dbg[:, :], f2co[:, :])
        # ---- per-core f1 slice WITHOUT the AllToAll ----
        # whTo is identical on every core after the AllReduce, so the
        # per-core column slice of f1 = ao1^T WhoT is obtained by a one-hot
        # selection matmul against the per-core input psel [N, ISL]
        # (psel[c*ISL+i, i] = 1): f1sl = sum_jb f1col[:, jb]^T @ psel_jb.
        # f1 in column layout first (same shape machinery as f2co):
        f1cps = b.ps_sm.tile([128, NJB], FP32, tag="sm", name="sm")
        for jb in range(NJB):
            nc.tensor.matmul(
                f1cps[:, jb : jb + 1],
                whTo[:, jb * 128 : (jb + 1) * 128],
                ao1_sb[:, :],
                start=True, stop=True,
            )
        f1colb = b.state.tile([128, NJB], BF16, tag="f1colb")
        nc.scalar.activation(f1colb[:, :], f1cps[:, :], AF.Copy)
        f1sps = b.ps_sm.tile([1, 512], FP32, tag="sm", name="sm")
        for jb in range(NJB):
            pt = b.psel.tile([128, ISL], BF16, tag="psel")
            nc.sync.dma_start(pt[:, :],
                              psel[jb * 128 : (jb + 1) * 128, :])
            nc.tensor.matmul(
                f1sps[:, :ISL], f1colb[:, jb : jb + 1], pt[:, :],
                start=(jb == 0), stop=(jb == NJB - 1),
            )
        f1slrow = b.state.tile([1, ISL], BF16, tag="f1slrow")
        nc.scalar.activation(f1slrow[:, :], f1sps[:, :ISL], AF.Copy)
        f1bo = b.bcast_row(f1slrow, ISL, "f1bc")
        pbo = b.bcast_row(f1slrow, ISL, "p_bc", exp_scale=ALPHA)
        o_fin = b.state.tile([OUT, ISL], FP32, tag="o_fin")
        b.attention_agg(adjT_osl, f2co, qco, f1bo, pbo, whnmo, OUT, ISL,
                        o_fin, out_elu=False, fold_rowsum=False)
        if dbg:
            nc.sync.dma_start(oraw_dbg[:, :], o_fin[:, :])
        # final ELU
        m = b.misc.tile([OUT, ISL], FP32, tag="fin_m")
        nc.vector.tensor_scalar(m[:, :], o_fin[:, :], 0.0, None, ALU.min)
        e = b.misc.tile([OUT, ISL], FP32, tag="fin_e")
        nc.scalar.activation(e[:, :], m[:, :], AF.Exp)
        r = b.misc.tile([OUT, ISL], FP32, tag="fin_r")
        nc.vector.tensor_scalar(r[:, :], o_fin[:, :], 0.0, -1.0, ALU.max,
                                ALU.add)
        fin = b.misc.tile([OUT, ISL], FP32, tag="fin")
        nc.vector.tensor_add(fin[:, :], e[:, :], r[:, :])
        nc.sync.dma_start(outT[:, :], fin[:, :])
    nc.compile()
    return nc


def make_in_maps(inputs):
    x = np.asarray(inputs["x"], np.float32)
    adj = np.asarray(inputs["adj"], np.float32)
    observation = np.asarray(inputs["observation"])
    theta = np.asarray(inputs["theta"], np.float32)
    W0 = np.asarray(inputs["W0"], np.float32)
    a0 = np.asarray(inputs["a0"], np.float32)
    W1 = np.asarray(inputs["W1"], np.float32)
    a1 = np.asarray(inputs["a1"], np.float32)
    Wo = np.asarray(inputs["Wo"], np.float32)
    ao = np.asarray(inputs["ao"], np.float32)

    bf = ml_dtypes.bfloat16
    xT = np.ascontiguousarray(x.T).astype(bf)
    seed = (observation[0] == 1).astype(np.float32)[None, :].astype(bf)
    theta_colh = np.ascontiguousarray(theta.reshape(F, 1)).astype(bf)
    adjT = np.ascontiguousarray((adj > 0).T.astype(bf))
    adjT0h = np.ascontiguousarray(adjT[:, :HALF])
    adjT1h = np.ascontiguousarray(adjT[:, HALF:])
    wo_bf = Wo.astype(bf)
    ident_bf = np.eye(128, dtype=np.float32).astype(bf)
    ao1 = np.ascontiguousarray(ao[:OUT]).astype(bf)
    ao2 = np.ascontiguousarray(ao[OUT:]).astype(bf)

    in_maps = []
    for c in range(NCORES):
        psel_c = np.zeros((N, ISL), np.float32)
        psel_c[c * ISL + np.arange(ISL), np.arange(ISL)] = 1.0
        in_maps.append({
            "psel": psel_c.astype(bf),
            "xT": xT, "seed": seed, "theta": theta_colh,
            "adjT0": adjT0h, "adjT1": adjT1h,
            "adjT_osl": np.ascontiguousarray(adjT[:, c * ISL : (c + 1) * ISL]),
            "w0": W0[c].astype(bf),
            "a01": (W0[c] @ a0[c][:D]).astype(bf),
            "a02": (W0[c] @ a0[c][D:]).astype(bf),
            "c01": np.array([[float((theta @ (W0[c] @ a0[c][:D])).item()),
                              float((theta @ (W0[c] @ a0[c][D:])).item())]],
                            np.float32).astype(bf),
            "w1": W1[c].astype(bf),
            "a11": (W1[c] @ a1[c][:D]).astype(bf),
            "a12": (W1[c] @ a1[c][D:]).astype(bf),
            "wo": np.ascontiguousarray(wo_bf[c * D : (c + 1) * D]),
            "ident": ident_bf, "ao1": ao1, "ao2": ao2,
        })
    return in_maps


def kernel(**inputs):
    in_maps = make_in_maps(inputs)
    nc = build()
    res = run_bass_kernel_spmd(nc, in_maps, core_ids=list(range(NCORES)))
    out = np.concatenate(
        [res.results[c]["outT"].T for c in range(NCORES)], axis=0
    )
    return np.ascontiguousarray(out, np.float32)


if __name__ == "__main__":
    build()
    print("built ok")



# revision 7
# speedup vs baseline: 1.2650x; 1.2650x over previous
"""Trainium2 Bass kernel for nn_GAT_38989713113447 (3-layer dense GAT).

Sharding: 8 heads over 8 cores for the two inner GAT layers (pure head
parallelism, no communication).  The head-concat + output projection
commutes into a sum of per-head projections: Who = sum_k h1_k @ Wo[k],
so a small AllReduce of [64, N] replaces an AllGather of the full
[1024, N] concat.  The output attention layer is sharded over node rows
(384 rows/core); the per-core column slice of WhoT is selected via a
one-hot matmul against a per-core input (no AllToAll), keeping the SPMD
program identical on every core.  The final [3072, 64] output is
assembled host-side from the per-core row slices.

Attention math (per column i, softmax over j):  the per-column factor
exp(f1_i) cancels in softmax, so with
    at[j,i]  = m * max(exp(f1_i+f2_j), exp(a*f1_i + a*f2_j))      (a=0.2)
    at'[j,i] = at[j,i] / exp(f1_i)
             = m * max(exp(f2_j), exp((a-1)*f1_i + a*f2_j))
the first branch is per-PARTITION constant v_j = exp(f2_j).  One ACT
pass builds e1' = Exp((a-1)*f1bc + a*f2_j) (bias per-partition), one
DVE scalar_tensor_tensor builds at' = (e1' max v_j) * m.  That is 1 ACT
+ 1 DVE pass over [N,N] per layer (the unfactored form needs 2 DVE).

dtypes: masks and at' are fp8e4 (values O(1), exact {0,1} masks); the
adjacency mask is DMA'd once and stays SBUF-resident across all three
layers.  Aggregation matmuls run mixed lhsT=bf16(Wh) x rhs=fp8(at');
softmax denominators use fp8 DoubleRow matmuls (2 j-tiles per pass)
against an interleaved at-pair layout.  All HW-verified exact.
"""

import os
import sys

sys.path.insert(0, "/opt/trn_rl_repo")

from contextlib import ExitStack

import numpy as np
import ml_dtypes

import concourse.bass as bass  # noqa: F401
import concourse.bacc as bacc
import concourse.tile as tile
from concourse import mybir
from concourse.bass_utils import run_bass_kernel_spmd

N = 3072
F = 256
H = 8
D = 128          # H1 == H2
OUT = 64
ALPHA = 0.2
NCORES = 8
NJB = N // 128   # 24 attention j-blocks
HALF = N // 2    # i-dim half per PSUM residency
ISL = N // NCORES  # 384 output rows per core

FP32 = mybir.dt.float32
BF16 = mybir.dt.bfloat16
FP8 = mybir.dt.float8e4
AF = mybir.ActivationFunctionType
ALU = mybir.AluOpType

def _chunks(total, step):
    return [(o, min(step, total - o)) for o in range(0, total, step)]


class Builder:
    def __init__(self, nc, tc, ctx):
        self.nc = nc
        self.tc = tc
        p = lambda name, bufs, space=None: ctx.enter_context(
            tc.tile_pool(name=name, bufs=bufs, **({"space": space} if space else {}))
        )
        self.state = p("state", 1)
        self.mres = p("mres", 1)
        self.work = p("work", 4)
        self.att = p("att", 4)
        self.ps_agg = p("ps_agg", 1, "PSUM")
        self.ps_rs = p("ps_rs", 1, "PSUM")
        self.ps_sm = p("ps_sm", 2, "PSUM")
        self.misc = p("misc", 1)
        self.psel = p("psel", 2)

    def ones_tile(self, shape, dtype, name):
        t = self.state.tile(shape, dtype, tag=name, name=name)
        self.nc.vector.memset(t[:, :], 1.0)
        return t

    def ones_pair(self):
        """fp8 [128, 2, 1] stationary AP for DoubleRow rowsum (cols 0, 16)."""
        t = self.state.tile([128, 32], FP8, tag="ones_pair", name="ones_pair")
        nc = self.nc
        nc.vector.memset(t[:, :], 0.0)
        nc.vector.memset(t[:, 0:1], 1.0)
        nc.vector.memset(t[:, 16:17], 1.0)
        ap = t[:, :]
        return bass.AP(ap.tensor, ap.offset,
                       [list(ap.ap[0])] + [[16, 2], [1, 1]])

    def bcast_row(self, row_ap, width, tag):
        """[1, width] bf16 SBUF row -> [128, width] bf16 tile via a DMA with
        a partition-step-0 source AP (reads the row 128x)."""
        nc = self.nc
        if not hasattr(self, "_row_dram"):
            self._row_dram = {}
        if tag not in self._row_dram:
            self._row_dram[tag] = nc.dram_tensor(
                f"rowd_{tag}", [1, N], BF16)
        rd = self._row_dram[tag]
        out = self.state.tile([128, width], BF16, tag=tag)
        for off, w in _chunks(width, 512):
            nc.sync.dma_start(rd[:, off : off + w], row_ap[:, off : off + w])
            rs_ap = rd[:1, off : off + w]
            bsrc = bass.AP(rs_ap.tensor, rs_ap.offset,
                           [[0, 128]] + [list(p) for p in rs_ap.ap])
            nc.sync.dma_start(out[:, off : off + w], bsrc)
        return out

    def project(self, hT_tiles, w_sb, dT, rank1=None, want_whT=True,
                jb_range=None):
        """WhT[d, i] (feature-major) and node-major Wh_nm[jb][j, dT].

        rank1=(seed_row [1,N], thw_row [1,dT]): adds seed_i * thw_d (the
        mergeState theta-add folded through W) as one extra K=1 matmul."""
        nc = self.nc
        nk = len(hT_tiles)
        whT = None
        if want_whT:
            whT = self.state.tile([dT, N], BF16, tag="whT")
        for off, w in (_chunks(N, 512) if want_whT else []):
            ps = self.ps_sm.tile([dT, 512], FP32, tag="sm", name="sm")
            last_w = nk - 1 if rank1 is None else nk
            for k in range(nk):
                nc.tensor.matmul(
                    ps[:, :w], w_sb[k][:, :], hT_tiles[k][:, off : off + w],
                    start=(k == 0), stop=(k == last_w),
                )
            if rank1 is not None:
                seed_row, thw_row = rank1
                nc.tensor.matmul(
                    ps[:, :w], thw_row[:, :dT], seed_row[:, off : off + w],
                    start=False, stop=True,
                )
            nc.scalar.activation(whT[:, off : off + w], ps[:, :w], AF.Copy)
        wh_nm = []
        per = max(1, 512 // dT)  # jb blocks packed per PSUM tile
        jlo, jhi = jb_range if jb_range is not None else (0, NJB)
        for jb0 in range(jlo, jhi, per):
            njb = min(per, jhi - jb0)
            ps = self.ps_sm.tile([128, 512], FP32, tag="sm", name="sm")
            last_w = nk - 1 if rank1 is None else nk
            for u in range(njb):
                jb = jb0 + u
                for k in range(nk):
                    nc.tensor.matmul(
                        ps[:, u * dT : (u + 1) * dT],
                        hT_tiles[k][:, jb * 128 : (jb + 1) * 128],
                        w_sb[k][:, :],
                        start=(k == 0), stop=(k == last_w),
                    )
                if rank1 is not None:
                    seed_row, thw_row = rank1
                    nc.tensor.matmul(
                        ps[:, u * dT : (u + 1) * dT],
                        seed_row[:, jb * 128 : (jb + 1) * 128],
                        thw_row[:, :dT],
                        start=False, stop=True,
                    )
            t = self.state.tile([128, per * dT], BF16,
                                tag=f"whnm_{jb0 // per}", name="whnm")
            nc.scalar.activation(t[:, : njb * dT], ps[:, : njb * dT], AF.Copy)
            for u in range(njb):
                wh_nm.append(t[:, u * dT : (u + 1) * dT])
        return whT, wh_nm

    def f2_from_h(self, hT_tiles, wa2_sb, corr=None, tag="f2"):
        """f2col [128, NJB] fp32 via f2 = h @ (W a2); corr=(seed_row, c2)
        adds c2*seed_j (layer-0 theta fold).  Also returns
        vcol = exp(f2col) and af2col = ALPHA*f2col (ACT bias column)."""
        nc = self.nc
        nk = len(hT_tiles)
        f2ps = self.ps_sm.tile([128, NJB], FP32, tag="sm", name="sm")
        for jb in range(NJB):
            last = nk - 1 if corr is None else nk
            for k in range(nk):
                nc.tensor.matmul(
                    f2ps[:, jb : jb + 1],
                    hT_tiles[k][:, jb * 128 : (jb + 1) * 128],
                    wa2_sb[k][:, :],
                    start=(k == 0), stop=(k == last),
                )
            if corr is not None:
                seed_row, c2_tile = corr
                nc.tensor.matmul(
                    f2ps[:, jb : jb + 1],
                    seed_row[:, jb * 128 : (jb + 1) * 128],
                    c2_tile[:, :],
                    start=False, stop=True,
                )
        vcol = self.state.tile([128, NJB], FP32, tag=tag + "v")
        nc.scalar.activation(vcol[:, :], f2ps[:, :], AF.Exp)
        af2col = self.state.tile([128, NJB], FP32, tag=tag + "a")
        nc.scalar.activation(af2col[:, :], f2ps[:, :], AF.Copy, scale=ALPHA)
        return vcol, af2col

    def f1_from_h(self, hT_tiles, wa1_sb, width, corr=None):
        """f1bc [128, width] bf16 broadcast of f1 = h @ (W a1)."""
        nc = self.nc
        nk = len(hT_tiles)
        f1row = self.state.tile([1, width], BF16, tag="f1row")
        for off, w in _chunks(width, 512):
            ps = self.ps_sm.tile([1, 512], FP32, tag="sm", name="sm")
            last = nk - 1 if corr is None else nk
            for k in range(nk):
                nc.tensor.matmul(
                    ps[:, :w], wa1_sb[k][:, :],
                    hT_tiles[k][:, off : off + w],
                    start=(k == 0), stop=(k == last),
                )
            if corr is not None:
                seed_row, c1_tile = corr
                nc.tensor.matmul(
                    ps[:, :w], c1_tile[:, :], seed_row[:, off : off + w],
                    start=False, stop=True,
                )
            nc.scalar.activation(f1row[:, off : off + w], ps[:, :w], AF.Copy)
        return self.bcast_row(f1row, width, "f1bc")

    def attention_agg(self, mres, mcol, vcol, af2col, f1bc, wh_nm, dT,
                      width, h_out, out_elu, between=None):
        """Factored masked softmax + aggregation + normalize (+ELU).

        mres: list of NJB resident fp8 mask tiles [128, mask_width];
        mcol: column offset into them.  between: callback after the first
        i-half's emission (inputs must be fully emitted by then)."""
        nc = self.nc
        half_w = min(width, HALF)
        for h0 in range(0, width, half_w):
            if h0 > 0 and between is not None:
                between()
                between = None
            hw = min(half_w, width - h0)
            ch = _chunks(hw, 512)
            agg_ps = [self.ps_agg.tile([dT, 512], FP32, tag=f"agg{ci}",
                                       name=f"agg{ci}")
                      for ci in range(len(ch))]
            rs_ps = [self.ps_rs.tile([1, 512], FP32, tag=f"rs{ci}",
                                     name=f"rs{ci}")
                     for ci in range(len(ch))]
            npair = NJB // 2
            for pair in range(npair):
                atp = self.att.tile([128, 2 * hw], FP8, tag="atp")
                for side in range(2):
                    jb = 2 * pair + side
                    e1 = self.work.tile([128, hw], FP8, tag="e1")
                    nc.scalar.activation(
                        e1[:, :], f1bc[:, h0 : h0 + hw], AF.Exp,
                        scale=ALPHA - 1.0, bias=af2col[:, jb : jb + 1],
                    )
                    nc.vector.scalar_tensor_tensor(
                        atp[:, side * hw : (side + 1) * hw], e1[:, :],
                        vcol[:, jb : jb + 1],
                        mres[jb][:, mcol + h0 : mcol + h0 + hw],
                        ALU.max, ALU.mult,
                    )
                for side in range(2):
                    jb = 2 * pair + side
                    for ci, (off, w) in enumerate(ch):
                        nc.tensor.matmul(
                            agg_ps[ci][:, :w], wh_nm[jb],
                            atp[:, side * hw + off : side * hw + off + w],
                            start=(jb == 0), stop=(jb == NJB - 1),
                        )
                for ci, (off, w) in enumerate(ch):
                    base = atp[:, off : off + w]
                    rhs = bass.AP(base.tensor, base.offset,
                                  [list(base.ap[0])] + [[hw, 2], [1, w]])
                    nc.tensor.matmul(
                        rs_ps[ci][:, :w], self.ones_pair_ap, rhs,
                        start=(pair == 0), stop=(pair == npair - 1),
                        perf_mode=mybir.MatmulPerfMode.DoubleRow,
                    )
            # softmax denominator -> reciprocal -> broadcast -> normalize
            rinv = self.misc.tile([1, hw], FP32, tag="rinv")
            for ci, (off, w) in enumerate(ch):
                nc.vector.reciprocal_approx_fast(rinv[:, off : off + w],
                                                 rs_ps[ci][:, :w])
            rb_sb = self.misc.tile([dT, hw], BF16, tag="rb_sb")
            for off, w in _chunks(hw, 512):
                ps = self.ps_sm.tile([dT, 512], FP32, tag="sm", name="sm")
                nc.tensor.matmul(
                    ps[:, :w], self.ones1f[:, :dT], rinv[:, off : off + w],
                    start=True, stop=True,
                )
                nc.scalar.activation(rb_sb[:, off : off + w], ps[:, :w], AF.Copy)
            hpn = self.misc.tile([dT, hw], FP32 if not out_elu else BF16,
                                 tag="hpn")
            for ci, (off, w) in enumerate(ch):
                nc.vector.tensor_tensor(
                    hpn[:, off : off + w], agg_ps[ci][:dT, :w],
                    rb_sb[:, off : off + w], ALU.mult,
                )
            if out_elu:
                # ELU(x) = exp(min(x,0)) - 1 + max(x,0)
                m = self.misc.tile([dT, hw], BF16, tag="elu_m")
                nc.vector.tensor_scalar(m[:, :], hpn[:, :], 0.0, None, ALU.min)
                e = self.misc.tile([dT, hw], BF16, tag="elu_e")
                nc.scalar.activation(e[:, :], m[:, :], AF.Exp)
                r = self.misc.tile([dT, hw], BF16, tag="elu_m")
                nc.vector.tensor_scalar(
                    r[:, :], hpn[:, :], 0.0, -1.0, ALU.max, ALU.add
                )
                nc.vector.tensor_add(h_out[:, h0 : h0 + hw], e[:, :], r[:, :])
            else:
                nc.vector.tensor_copy(h_out[:, h0 : h0 + hw], hpn[:, :])


def build(dbg=False, sim=False):
    nc = bacc.Bacc("TRN2", target_bir_lowering=False, num_devices=NCORES)

    xT = nc.dram_tensor("xT", [F, N], BF16, kind="ExternalInput")
    seed = nc.dram_tensor("seed", [1, N], BF16, kind="ExternalInput")
    theta = nc.dram_tensor("theta", [F, 1], BF16, kind="ExternalInput")
    adjT = nc.dram_tensor("adjT", [N, N], FP8, kind="ExternalInput")
    adjT_osl = nc.dram_tensor("adjT_osl", [N, ISL], FP8, kind="ExternalInput")
    w0 = nc.dram_tensor("w0", [F, D], BF16, kind="ExternalInput")
    a01 = nc.dram_tensor("a01", [F, 1], BF16, kind="ExternalInput")
    a02 = nc.dram_tensor("a02", [F, 1], BF16, kind="ExternalInput")
    c01 = nc.dram_tensor("c01", [1, 2], BF16, kind="ExternalInput")
    w1 = nc.dram_tensor("w1", [D, D], BF16, kind="ExternalInput")
    a11 = nc.dram_tensor("a11", [D, 1], BF16, kind="ExternalInput")
    a12 = nc.dram_tensor("a12", [D, 1], BF16, kind="ExternalInput")
    wo = nc.dram_tensor("wo", [D, OUT], BF16, kind="ExternalInput")
    ident = nc.dram_tensor("ident", [128, 128], BF16, kind="ExternalInput")
    ao1 = nc.dram_tensor("ao1", [OUT, 1], BF16, kind="ExternalInput")
    ao2 = nc.dram_tensor("ao2", [OUT, 1], BF16, kind="ExternalInput")

    outT = nc.dram_tensor("outT", [OUT, ISL], FP32, kind="ExternalOutput")
    if dbg:
        h0_dbg = nc.dram_tensor("h0_dbg", [D, N], BF16, kind="ExternalOutput")
        h1_dbg = nc.dram_tensor("h1_dbg", [D, N], BF16, kind="ExternalOutput")

    ar_in = nc.dram_tensor("ar_in", [OUT, N], BF16)
    ar_out = nc.dram_tensor("ar_out", [OUT, N], BF16, addr_space="Shared")
    psel = nc.dram_tensor("psel", [N, ISL], FP8, kind="ExternalInput")

    with tile.TileContext(nc) as tc, ExitStack() as ctx:
        b = Builder(nc, tc, ctx)
        b.ones1f = b.ones_tile([1, 128], FP32, "ones1f")
        b.ones_pair_ap = b.ones_pair()

        def load_w(ap, shape, tag, dt=BF16):
            s = b.state.tile(shape, dt, tag=tag, name=tag)
            nc.sync.dma_start(s[:, :], ap)
            return s

        theta_col = [load_w(theta[k * 128 : (k + 1) * 128, :], [128, 1],
                            f"theta{k}") for k in range(F // 128)]
        seed_sb = load_w(seed[:, :], [1, N], "seed")
        w0_sb = [load_w(w0[k * 128 : (k + 1) * 128, :], [128, D], f"w0_{k}")
                 for k in range(F // 128)]
        wa01_sb = [load_w(a01[k * 128 : (k + 1) * 128, :], [128, 1],
                          f"wa01_{k}") for k in range(F // 128)]
        wa02_sb = [load_w(a02[k * 128 : (k + 1) * 128, :], [128, 1],
                          f"wa02_{k}") for k in range(F // 128)]
        c01_sb = load_w(c01[:, :], [1, 2], "c01")
        w1_sb = [load_w(w1[:, :], [D, D], "w1")]
        wa11_sb = [load_w(a11[:, :], [D, 1], "wa11")]
        wa12_sb = [load_w(a12[:, :], [D, 1], "wa12")]
        wo_sb = [load_w(wo[:, :], [D, OUT], "wo")]
        ident_sb = load_w(ident[:, :], [128, 128], "ident")
        ao1_sb = load_w(ao1[:, :], [OUT, 1], "ao1")
        ao2_sb = load_w(ao2[:, :], [OUT, 1], "ao2")

        # ---- layer 0 ----
        # mergeState folds through W0: (x + seed*theta) @ W0 =
        # x@W0 + seed x (theta@W0); no hT materialization needed.
        xT_sb = []
        for fb in range(F // 128):
            t = b.state.tile([128, N], BF16, tag=f"hT{fb}", name="xTsb")
            nc.sync.dma_start(t[:, :], xT[fb * 128 : (fb + 1) * 128, :])
            xT_sb.append(t)

        # resident fp8 masks (full adjT, all layers; per-core output slice;
        # psel one-hot).  Issued on the otherwise-idle Pool queue so the
        # sync queue (f1 broadcasts etc.) is never head-of-line blocked
        # behind this ~11MB stream.
        mres = []
        for jb in range(NJB):
            t = b.mres.tile([128, N], FP8, tag=f"mres{jb}", name="mres")
            nc.gpsimd.dma_start(t[:, :], adjT[jb * 128 : (jb + 1) * 128, :])
            mres.append(t)
        mosl = []
        for jb in range(NJB):
            t = b.mres.tile([128, ISL], FP8, tag=f"mosl{jb}", name="mosl")
            nc.gpsimd.dma_start(t[:, :],
                                adjT_osl[jb * 128 : (jb + 1) * 128, :])
            mosl.append(t)
        psel_sb = []
        for jb in range(NJB):
            t = b.mres.tile([128, ISL], FP8, tag=f"psel{jb}", name="psel")
            nc.gpsimd.dma_start(t[:, :], psel[jb * 128 : (jb + 1) * 128, :])
            psel_sb.append(t)
        thw_ps = b.ps_sm.tile([1, D], FP32, tag="sm", name="sm")
        for k in range(F // 128):
            nc.tensor.matmul(
                thw_ps[:, :], theta_col[k][:, :],
                w0_sb[k][:, :], start=(k == 0), stop=(k == F // 128 - 1),
            )
        thw_row = b.state.tile([1, D], BF16, tag="thw_row", name="thw_row")
        nc.scalar.activation(thw_row[:, :], thw_ps[:, :], AF.Copy)
        _, whnm0 = b.project(xT_sb, w0_sb, D, rank1=(seed_sb, thw_row),
                             want_whT=False)
        vc0, af20 = b.f2_from_h(xT_sb, wa02_sb,
                                corr=(seed_sb, c01_sb[:, 1:2]), tag="f2l0")
        f1b0 = b.f1_from_h(xT_sb, wa01_sb, N,
                           corr=(seed_sb, c01_sb[:, 0:1]))
        h0T = b.state.tile([D, N], BF16, tag="h0T")
        l1_pre = {}

        def emit_l1_early():
            # Safe early emission: jb 0-11 read only h0T columns < HALF,
            # whose writes are already emitted (Tile orders an
            # earlier-emitted read BEFORE a later-emitted write).
            _, l1_pre["whnm_a"] = b.project([h0T], w1_sb, D, want_whT=False,
                                            jb_range=(0, NJB // 2))

        b.attention_agg(mres, 0, vc0, af20, f1b0, whnm0, D, N,
                        h0T, out_elu=True, between=emit_l1_early)

        # ---- layer 1 ----
        _, whnm1_b = b.project([h0T], w1_sb, D, want_whT=False,
                               jb_range=(NJB // 2, NJB))
        whnm1 = l1_pre["whnm_a"] + whnm1_b
        vc1, af21 = b.f2_from_h([h0T], wa12_sb, tag="f2l1")
        f1b1 = b.f1_from_h([h0T], wa11_sb, N)
        h1T = b.state.tile([D, N], BF16, tag="h1T")
        who_part = b.state.tile([OUT, N], BF16, tag="who_part")

        def _who_chunk(off, w):
            ps = b.ps_sm.tile([OUT, 512], FP32, tag="sm", name="sm")
            nc.tensor.matmul(
                ps[:, :w], wo_sb[0][:, :], h1T[:, off : off + w],
                start=True, stop=True,
            )
            nc.scalar.activation(who_part[:, off : off + w], ps[:, :w],
                                 AF.Copy)
            nc.sync.dma_start(ar_in[:, off : off + w],
                              who_part[:, off : off + w])

        def emit_who_early():
            # h1T cols < HALF are fully emitted after half 0; stage the
            # first half of the AllReduce input early.
            for off, w in _chunks(HALF, 512):
                _who_chunk(off, w)

        b.attention_agg(mres, 0, vc1, af21, f1b1, whnm1, D, N,
                        h1T, out_elu=True, between=emit_who_early)

        if dbg:
            nc.sync.dma_start(h0_dbg[:, :], h0T[:, :])
            nc.sync.dma_start(h1_dbg[:, :], h1T[:, :])

        # ---- out layer via AllReduce of per-head Who contributions ----
        # Who = hc @ Wo = sum_k h1_k @ Wo[k-block]; each core computes its
        # head's [64, N] share locally; AllReduce(add) sums over heads.
        for off, w in _chunks(N, 512):
            if off < HALF:
                continue  # emitted early between layer-1 halves
            _who_chunk(off, w)
        if sim or os.environ.get("GAT_NO_COLL"):
            nc.sync.dma_start(ar_out[:, :], ar_in[:, :])
        else:
            nc.gpsimd.collective_compute(
                "AllReduce", ALU.add,
                replica_groups=[list(range(NCORES))],
                ins=[ar_in.ap().opt()], outs=[ar_out.ap().opt()],
            )
        whTo = b.state.tile([OUT, N], BF16, tag="whTo")
        nc.sync.dma_start(whTo[:, :], ar_out[:, :])
        # node-major Who via PE transpose of WhoT 128-col chunks
        whnmo = []
        for jb in range(NJB):
            ps = b.ps_sm.tile([128, OUT], BF16, tag="sm", name="sm")
            nc.tensor.transpose(
                ps[:, :], whTo[:, jb * 128 : (jb + 1) * 128],
                ident_sb[:OUT, :OUT],
            )
            t = b.state.tile([128, OUT], BF16, tag=f"whnm_{jb}", name="whnm")
            nc.vector.tensor_copy(t[:, :], ps[:, :])
            whnmo.append(t)
        # f2 column vector for the out layer (from whTo, K=OUT)
        f2ps = b.ps_sm.tile([128, NJB], FP32, tag="sm", name="sm")
        for jb in range(NJB):
            nc.tensor.matmul(
                f2ps[:, jb : jb + 1],
                whTo[:, jb * 128 : (jb + 1) * 128], ao2_sb[:, :],
                start=True, stop=True,
            )
        vco = b.state.tile([128, NJB], FP32, tag="f2ov")
        nc.scalar.activation(vco[:, :], f2ps[:, :], AF.Exp)
        af2o = b.state.tile([128, NJB], FP32, tag="f2oa")
        nc.scalar.activation(af2o[:, :], f2ps[:, :], AF.Copy, scale=ALPHA)
        # ---- per-core f1 slice WITHOUT the AllToAll ----
        # whTo is identical on every core after the AllReduce, so the
        # per-core column slice of f1 = ao1^T WhoT is obtained by a one-hot
        # selection matmul against the per-core input psel [N, ISL]
        # (psel[c*ISL+i, i] = 1): f1sl = sum_jb f1col[:, jb]^T @ psel_jb.
        f1cps = b.ps_sm.tile([128, NJB], FP32, tag="sm", name="sm")
        for jb in range(NJB):
            nc.tensor.matmul(
                f1cps[:, jb : jb + 1],
                whTo[:, jb * 128 : (jb + 1) * 128],
                ao1_sb[:, :],
                start=True, stop=True,
            )
        f1colb = b.state.tile([128, NJB], BF16, tag="f1colb")
        nc.scalar.activation(f1colb[:, :], f1cps[:, :], AF.Copy)
        f1sps = b.ps_sm.tile([1, 512], FP32, tag="sm", name="sm")
        for jb in range(NJB):
            nc.tensor.matmul(
                f1sps[:, :ISL], f1colb[:, jb : jb + 1], psel_sb[jb][:, :],
                start=(jb == 0), stop=(jb == NJB - 1),
            )
        f1slrow = b.state.tile([1, ISL], BF16, tag="f1slrow")
        nc.scalar.activation(f1slrow[:, :], f1sps[:, :ISL], AF.Copy)
        f1bo = b.bcast_row(f1slrow, ISL, "f1bc_o")
        o_fin = b.state.tile([OUT, ISL], FP32, tag="o_fin")
        b.attention_agg(mosl, 0, vco, af2o, f1bo, whnmo, OUT, ISL,
                        o_fin, out_elu=False)
        # final ELU
        m = b.misc.tile([OUT, ISL], FP32, tag="fin_m")
        nc.vector.tensor_scalar(m[:, :], o_fin[:, :], 0.0, None, ALU.min)
        e = b.misc.tile([OUT, ISL], FP32, tag="fin_e")
        nc.scalar.activation(e[:, :], m[:, :], AF.Exp)
        r = b.misc.tile([OUT, ISL], FP32, tag="fin_r")
        nc.vector.tensor_scalar(r[:, :], o_fin[:, :], 0.0, -1.0, ALU.max,
                                ALU.add)
        fin = b.misc.tile([OUT, ISL], FP32, tag="fin")
        nc.vector.tensor_add(fin[:, :], e[:, :], r[:, :])
        nc.sync.dma_start(outT[:, :], fin[:, :])
    nc.compile()
    return nc


def make_in_maps(inputs):
    x = np.asarray(inputs["x"], np.float32)
    adj = np.asarray(inputs["adj"], np.float32)
    observation = np.asarray(inputs["observation"])
    theta = np.asarray(inputs["theta"], np.float32)
    W0 = np.asarray(inputs["W0"], np.float32)
    a0 = np.asarray(inputs["a0"], np.float32)
    W1 = np.asarray(inputs["W1"], np.float32)
    a1 = np.asarray(inputs["a1"], np.float32)
    Wo = np.asarray(inputs["Wo"], np.float32)
    ao = np.asarray(inputs["ao"], np.float32)

    bf = ml_dtypes.bfloat16
    f8 = ml_dtypes.float8_e4m3
    xT = np.ascontiguousarray(x.T).astype(bf)
    seed = (observation[0] == 1).astype(np.float32)[None, :].astype(bf)
    theta_colh = np.ascontiguousarray(theta.reshape(F, 1)).astype(bf)
    adjT = np.ascontiguousarray((adj > 0).T.astype(np.float32)).astype(f8)
    wo_bf = Wo.astype(bf)
    ident_bf = np.eye(128, dtype=np.float32).astype(bf)
    ao1 = np.ascontiguousarray(ao[:OUT]).astype(bf)
    ao2 = np.ascontiguousarray(ao[OUT:]).astype(bf)

    in_maps = []
    for c in range(NCORES):
        psel_c = np.zeros((N, ISL), np.float32)
        psel_c[c * ISL + np.arange(ISL), np.arange(ISL)] = 1.0
        in_maps.append({
            "psel": psel_c.astype(f8),
            "xT": xT, "seed": seed, "theta": theta_colh,
            "adjT": adjT,
            "adjT_osl": np.ascontiguousarray(adjT[:, c * ISL : (c + 1) * ISL]),
            "w0": W0[c].astype(bf),
            "a01": (W0[c] @ a0[c][:D]).astype(bf),
            "a02": (W0[c] @ a0[c][D:]).astype(bf),
            "c01": np.array([[float((theta @ (W0[c] @ a0[c][:D])).item()),
                              float((theta @ (W0[c] @ a0[c][D:])).item())]],
                            np.float32).astype(bf),
            "w1": W1[c].astype(bf),
            "a11": (W1[c] @ a1[c][:D]).astype(bf),
            "a12": (W1[c] @ a1[c][D:]).astype(bf),
            "wo": np.ascontiguousarray(wo_bf[c * D : (c + 1) * D]),
            "ident": ident_bf, "ao1": ao1, "ao2": ao2,
        })
    return in_maps


def kernel(**inputs):
    in_maps = make_in_maps(inputs)
    nc = build()
    res = run_bass_kernel_spmd(nc, in_maps, core_ids=list(range(NCORES)))
    out = np.concatenate(
        [res.results[c]["outT"].T for c in range(NCORES)], axis=0
    )
    return np.ascontiguousarray(out, np.float32)


if __name__ == "__main__":
    build()
    print("built ok")
